# revision 6
# baseline (speedup 1.0000x reference)
"""HGT layer kernel for Trainium2, 8 NeuronCores (SPMD via bass/Tile).

Strategy (per spec sharding hint): shard edges/nnz across 8 cores by
destination (entity for entity_agg, user for user_agg) so every segment
reduction completes on one core; replicate entity_emb and the small
weight/relation tensors.

Per core, three phases:
  A (relation-major): indirect-gather emb rows for tail/head, PE-transpose,
    q/k/v matmuls (relation matrices pre-folded into W_K/W_V block-diagonals
    on host), score -> exp, u' = [v*ex | ex] staged to HBM.
  B (destination-major): indirect-gather u' rows by slot, one-hot scatter
    matmul into 128-node PSUM windows (numerator + denominator together),
    then divide + W_O per window.
  U (user): indirect-gather emb rows, one-hot (x im_vals) scatter matmul
    into 128-user PSUM windows.
"""

import math

import numpy as np

# --- problem constants (hardcoded; kernel.py must be self-contained) ---
N = 100000
C = 128
E = 600000
NNZ = 1000000
NU = 50000
NREL = 24
H = 4
DK = 32
NCORES = 8

P = 128
G = 8                 # pass-A tiles per gather group
TR = 28               # pass-A tiles per relation (capacity 3584 edges)
TA = NREL * TR        # 672 pass-A tiles
EA = TA * P           # 86016 pass-A slots
NWIN = 100            # entity node windows per core (12800 nodes)
TW = 8                # pass-B tiles per node window (capacity 1024 edges)
TB = NWIN * TW        # 800 pass-B tiles
UPC = NU // NCORES    # 6250 users per core
UWIN = 49             # user windows per core (6272 users >= 6250)
TU = 23               # user tiles per window (capacity 2944 nnz)
TUT = UWIN * TU       # 1127 user tiles
UC = 132              # u' row: 128 weighted-v + 4 ex columns

_CACHE = {}


def _build_program():
    import concourse.bass as bass
    import concourse.tile as tile
    from concourse import bacc, mybir
    from concourse.masks import make_identity

    f32 = mybir.dt.float32
    i32 = mybir.dt.int32

    nc = bacc.Bacc("TRN2", target_bir_lowering=False, debug=False,
                   num_devices=NCORES)

    emb = nc.dram_tensor("emb", [N, C], f32, kind="ExternalInput")
    wkv = nc.dram_tensor("wkv", [NREL * C, 2 * C], f32, kind="ExternalInput")
    wq = nc.dram_tensor("wq", [C, C], f32, kind="ExternalInput")
    wo = nc.dram_tensor("wo", [C, C], f32, kind="ExternalInput")
    tailT = nc.dram_tensor("tailT", [P, TA], i32, kind="ExternalInput")
    headT = nc.dram_tensor("headT", [P, TA], i32, kind="ExternalInput")
    biasT = nc.dram_tensor("biasT", [P, TA], f32, kind="ExternalInput")
    uslotT = nc.dram_tensor("uslotT", [P, TB], i32, kind="ExternalInput")
    lobT = nc.dram_tensor("lobT", [P, TB], f32, kind="ExternalInput")
    ucolT = nc.dram_tensor("ucolT", [P, TUT], i32, kind="ExternalInput")
    uvalT = nc.dram_tensor("uvalT", [P, TUT], f32, kind="ExternalInput")
    ulouT = nc.dram_tensor("ulouT", [P, TUT], f32, kind="ExternalInput")

    out_entT = nc.dram_tensor("out_entT", [P, NWIN * P], f32,
                              kind="ExternalOutput")
    out_user = nc.dram_tensor("out_user", [UWIN * P, C], f32,
                              kind="ExternalOutput")

    u_stage = nc.dram_tensor("u_stage", [EA, UC], f32)

    mult = mybir.AluOpType.mult
    addop = mybir.AluOpType.add
    iseq = mybir.AluOpType.is_equal
    AX = mybir.AxisListType.X
    EXP = mybir.ActivationFunctionType.Exp

    def bc(ap, n):
        """broadcast a [128, k] AP to [128, k, n] with a step-0 inner dim."""
        return bass.AP(ap.tensor, ap.offset, ap.ap + [[0, n]])

    with tile.TileContext(nc) as tc:
        with tc.tile_pool(name="const", bufs=1) as cpool:
            wkv_sb = cpool.tile([P, NREL * 2 * C], f32)
            for r in range(NREL):
                nc.sync.dma_start(wkv_sb[:, r * 2 * C:(r + 1) * 2 * C],
                                  wkv[r * C:(r + 1) * C, :])
            wq_sb = cpool.tile([P, C], f32)
            nc.sync.dma_start(wq_sb[:], wq[:])
            wo_sb = cpool.tile([P, C], f32)
            nc.sync.dma_start(wo_sb[:], wo[:])
            ident = cpool.tile([P, P], f32)
            make_identity(nc, ident[:])
            iota_i = cpool.tile([P, P], i32)
            nc.gpsimd.iota(iota_i[:], pattern=[[1, P]], base=0,
                           channel_multiplier=0)
            iota_f = cpool.tile([P, P], f32)
            nc.vector.tensor_copy(iota_f[:], iota_i[:])
            tailT_sb = cpool.tile([P, TA], i32)
            nc.sync.dma_start(tailT_sb[:], tailT[:])
            headT_sb = cpool.tile([P, TA], i32)
            nc.sync.dma_start(headT_sb[:], headT[:])
            biasT_sb = cpool.tile([P, TA], f32)
            nc.sync.dma_start(biasT_sb[:], biasT[:])
            uslotT_sb = cpool.tile([P, TB], i32)
            nc.sync.dma_start(uslotT_sb[:], uslotT[:])
            lobT_sb = cpool.tile([P, TB], f32)
            nc.sync.dma_start(lobT_sb[:], lobT[:])
            ucolT_sb = cpool.tile([P, TUT], i32)
            nc.sync.dma_start(ucolT_sb[:], ucolT[:])
            uvalT_sb = cpool.tile([P, TUT], f32)
            nc.sync.dma_start(uvalT_sb[:], uvalT[:])
            ulouT_sb = cpool.tile([P, TUT], f32)
            nc.sync.dma_start(ulouT_sb[:], ulouT[:])
            entT_sb = cpool.tile([P, NWIN * P], f32)

            scale = 1.0 / math.sqrt(DK)

            # ---------------- pass A ----------------
            with tc.tile_pool(name="pa_g", bufs=2) as pg, \
                 tc.tile_pool(name="pa_u", bufs=2) as pu, \
                 tc.tile_pool(name="pa_w", bufs=3) as pw, \
                 tc.tile_pool(name="pa_s", bufs=3) as ps, \
                 tc.tile_pool(name="pa_ps", bufs=2, space="PSUM") as pps, \
                 tc.tile_pool(name="pa_ps2", bufs=2, space="PSUM") as pps2:
                for g in range(TA // G):
                    ug = pu.tile([P, G, UC], f32, tag="ug")
                    for j in range(G):
                        t = g * G + j
                        r = t // TR
                        gt = pg.tile([P, C], f32, tag="gt")
                        nc.gpsimd.indirect_dma_start(
                            out=gt[:], out_offset=None, in_=emb[:],
                            in_offset=bass.IndirectOffsetOnAxis(
                                ap=tailT_sb[:, t:t + 1], axis=0))
                        gh = pg.tile([P, C], f32, tag="gh")
                        nc.gpsimd.indirect_dma_start(
                            out=gh[:], out_offset=None, in_=emb[:],
                            in_offset=bass.IndirectOffsetOnAxis(
                                ap=headT_sb[:, t:t + 1], axis=0))
                        ptt = pps.tile([P, P], f32, tag="ptt")
                        nc.tensor.transpose(ptt[:], gt[:], ident[:])
                        ett = pw.tile([P, P], f32, tag="ett")
                        nc.scalar.copy(ett[:], ptt[:])
                        pth = pps.tile([P, P], f32, tag="pth")
                        nc.tensor.transpose(pth[:], gh[:], ident[:])
                        eth = pw.tile([P, P], f32, tag="eth")
                        nc.vector.tensor_copy(eth[:], pth[:])
                        pkv = pps2.tile([P, 2 * C], f32, tag="pkv")
                        nc.tensor.matmul(pkv[:], lhsT=ett[:],
                                         rhs=wkv_sb[:, r * 2 * C:(r + 1) * 2 * C],
                                         start=True, stop=True)
                        pq = pps2.tile([P, C], f32, tag="pq")
                        nc.tensor.matmul(pq[:], lhsT=eth[:], rhs=wq_sb[:],
                                         start=True, stop=True)
                        ksb = pw.tile([P, C], f32, tag="ksb")
                        nc.scalar.copy(ksb[:], pkv[:, 0:C])
                        qk = ps.tile([P, C], f32, tag="qk")
                        nc.vector.tensor_tensor(qk[:], pq[:], ksb[:], op=mult)
                        sc = ps.tile([P, H], f32, tag="sc")
                        nc.vector.tensor_reduce(
                            sc[:], qk[:].rearrange("p (h d) -> p h d", d=DK),
                            axis=AX, op=addop)
                        nc.scalar.activation(ug[:, j, C:UC], sc[:], EXP,
                                             bias=biasT_sb[:, t:t + 1],
                                             scale=scale)
                        nc.vector.tensor_tensor(
                            ug[:, j, 0:C].rearrange("p (h d) -> p h d", d=DK),
                            pkv[:, C:2 * C].rearrange("p (h d) -> p h d", d=DK),
                            bc(ug[:, j, C:UC], DK), op=mult)
                    nc.sync.dma_start(
                        u_stage[g * G * P:(g + 1) * G * P, :].rearrange(
                            "(j p) c -> p j c", p=P), ug[:])

            # ---------------- pass B ----------------
            with tc.tile_pool(name="pb_g", bufs=2) as pbg, \
                 tc.tile_pool(name="pb_s", bufs=3) as pbs, \
                 tc.tile_pool(name="pb_w", bufs=2) as pbw, \
                 tc.tile_pool(name="pb_ps", bufs=2, space="PSUM") as pbps, \
                 tc.tile_pool(name="pb_ps2", bufs=2, space="PSUM") as pbps2:
                for w in range(NWIN):
                    ub = pbg.tile([P, TW, UC], f32, tag="ub")
                    for j in range(TW):
                        nc.gpsimd.indirect_dma_start(
                            out=ub[:, j, :], out_offset=None, in_=u_stage[:],
                            in_offset=bass.IndirectOffsetOnAxis(
                                ap=uslotT_sb[:, w * TW + j:w * TW + j + 1],
                                axis=0))
                    pwin = pbps.tile([P, UC], f32, tag="pwin")
                    for j in range(TW):
                        S = pbs.tile([P, P], f32, tag="S")
                        nc.vector.tensor_scalar(
                            S[:], iota_f[:],
                            lobT_sb[:, w * TW + j:w * TW + j + 1], None,
                            op0=iseq)
                        nc.tensor.matmul(pwin[:], lhsT=S[:], rhs=ub[:, j, :],
                                         start=(j == 0), stop=(j == TW - 1))
                    den = pbw.tile([P, H], f32, tag="den")
                    nc.vector.tensor_scalar(den[:], pwin[:, C:UC], 1e-30,
                                            None, op0=addop)
                    rec = pbw.tile([P, H], f32, tag="rec")
                    nc.vector.reciprocal(rec[:], den[:])
                    xn = pbw.tile([P, C], f32, tag="xn")
                    nc.vector.tensor_tensor(
                        xn[:].rearrange("p (h d) -> p h d", d=DK),
                        pwin[:, 0:C].rearrange("p (h d) -> p h d", d=DK),
                        bc(rec[:], DK), op=mult)
                    ptr = pbps2.tile([P, P], f32, tag="ptr")
                    nc.tensor.transpose(ptr[:], xn[:], ident[:])
                    xt = pbw.tile([P, C], f32, tag="xt")
                    nc.scalar.copy(xt[:], ptr[:])
                    po = pbps2.tile([P, C], f32, tag="po")
                    nc.tensor.matmul(po[:], lhsT=wo_sb[:], rhs=xt[:],
                                     start=True, stop=True)
                    nc.vector.tensor_copy(entT_sb[:, w * P:(w + 1) * P], po[:])
                nc.sync.dma_start(out_entT[:], entT_sb[:])

            # ---------------- user phase ----------------
            with tc.tile_pool(name="uu_g", bufs=2) as uug, \
                 tc.tile_pool(name="uu_s", bufs=3) as uus, \
                 tc.tile_pool(name="uu_o", bufs=2) as uuo, \
                 tc.tile_pool(name="uu_ps", bufs=2, space="PSUM") as uups:
                for w in range(UWIN):
                    gu = uug.tile([P, TU, C], f32, tag="gu")
                    for j in range(TU):
                        nc.gpsimd.indirect_dma_start(
                            out=gu[:, j, :], out_offset=None, in_=emb[:],
                            in_offset=bass.IndirectOffsetOnAxis(
                                ap=ucolT_sb[:, w * TU + j:w * TU + j + 1],
                                axis=0))
                    pusr = uups.tile([P, C], f32, tag="pusr")
                    for j in range(TU):
                        S = uus.tile([P, P], f32, tag="Su")
                        nc.vector.tensor_scalar(
                            S[:], iota_f[:],
                            ulouT_sb[:, w * TU + j:w * TU + j + 1],
                            uvalT_sb[:, w * TU + j:w * TU + j + 1],
                            op0=iseq, op1=mult)
                        nc.tensor.matmul(pusr[:], lhsT=S[:], rhs=gu[:, j, :],
                                         start=(j == 0), stop=(j == TU - 1))
                    ou = uuo.tile([P, C], f32, tag="ou")
                    nc.vector.tensor_copy(ou[:], pusr[:])
                    nc.sync.dma_start(out_user[w * P:(w + 1) * P, :], ou[:])

    nc.compile()
    return nc


def _prep(entity_emb, W_Q, W_K, W_V, W_O, relation_att, relation_msg,
          edge_index, edge_type, im_rows, im_cols, im_vals):
    """Host-side sharding/index prep. Returns (in_maps, bases, widths)."""
    f = np.float32

    # fold relation matrices into block-diagonal combined weights
    bd_att = np.zeros((NREL, C, C), f)
    bd_msg = np.zeros((NREL, C, C), f)
    for h in range(H):
        s = h * DK
        bd_att[:, s:s + DK, s:s + DK] = relation_att[:, h]
        bd_msg[:, s:s + DK, s:s + DK] = relation_msg[:, h]
    wk_r = np.einsum("ck,rkd->rcd", W_K.astype(f), bd_att)
    wv_r = np.einsum("ck,rkd->rcd", W_V.astype(f), bd_msg)
    wkv = np.concatenate([wk_r, wv_r], axis=2).reshape(NREL * C, 2 * C)
    wkv = np.ascontiguousarray(wkv)

    head = np.asarray(edge_index[0], np.int64)
    tail = np.asarray(edge_index[1], np.int64)
    rel = np.asarray(edge_type, np.int64) - 1
    order = np.argsort(head, kind="stable")
    hs, ts, rs = head[order], tail[order], rel[order]

    # core boundaries at node boundaries, balancing edge counts
    bases, splits = [0], [0]
    for c in range(1, NCORES):
        b = int(hs[min(c * (E // NCORES), E - 1)])
        bases.append(b)
        splits.append(int(np.searchsorted(hs, b, side="left")))
    bases.append(N)
    splits.append(E)

    in_maps = []
    widths = []
    for c in range(NCORES):
        lo_e, hi_e = splits[c], splits[c + 1]
        base = bases[c]
        width = bases[c + 1] - base
        widths.append(width)
        n_c = hi_e - lo_e
        ch, ct, cr = hs[lo_e:hi_e], ts[lo_e:hi_e], rs[lo_e:hi_e]
        assert width <= NWIN * P, f"core {c}: node width {width}"
        assert n_c <= EA

        # pass-A slots: group by relation
        cnt_r = np.bincount(cr, minlength=NREL)
        assert cnt_r.max() <= TR * P - 1, f"core {c}: rel count {cnt_r.max()}"
        ord_r = np.argsort(cr, kind="stable")
        rank_r = np.empty(n_c, np.int64)
        rank_r[ord_r] = np.arange(n_c) - np.concatenate(
            [[0], np.cumsum(cnt_r)])[cr[ord_r]]
        slot_a = cr * (TR * P) + rank_r

        tail_a = np.zeros(EA, np.int32)
        head_a = np.zeros(EA, np.int32)
        bias_a = np.full(EA, -1e30, f)
        tail_a[slot_a] = ct
        head_a[slot_a] = ch
        bias_a[slot_a] = 0.0

        # pass-B slots: group by 128-node window (edges already head-sorted)
        wloc = (ch - base) // P
        lob = ((ch - base) % P).astype(f)
        cnt_w = np.bincount(wloc, minlength=NWIN)
        assert cnt_w.max() <= TW * P, f"core {c}: window count {cnt_w.max()}"
        rank_w = np.arange(n_c) - np.concatenate([[0], np.cumsum(cnt_w)])[wloc]
        slot_b = wloc * (TW * P) + rank_w

        pad_slot = int(cnt_r[0])  # a pass-A padding slot (ex == 0 there)
        slot_arr = np.full(TB * P, pad_slot, np.int32)
        lob_arr = np.zeros(TB * P, f)
        slot_arr[slot_b] = slot_a
        lob_arr[slot_b] = lob

        in_maps.append({
            "tailT": np.ascontiguousarray(tail_a.reshape(TA, P).T),
            "headT": np.ascontiguousarray(head_a.reshape(TA, P).T),
            "biasT": np.ascontiguousarray(bias_a.reshape(TA, P).T),
            "uslotT": np.ascontiguousarray(slot_arr.reshape(TB, P).T),
            "lobT": np.ascontiguousarray(lob_arr.reshape(TB, P).T),
        })

    # user phase
    rows = np.asarray(im_rows, np.int64)
    cols = np.asarray(im_cols, np.int64)
    vals = np.asarray(im_vals, f)
    order_u = np.argsort(rows, kind="stable")
    ru, cu, vu = rows[order_u], cols[order_u], vals[order_u]
    for c in range(NCORES):
        lo_i = int(np.searchsorted(ru, c * UPC, side="left"))
        hi_i = int(np.searchsorted(ru, (c + 1) * UPC, side="left"))
        n_u = hi_i - lo_i
        rw = (ru[lo_i:hi_i] - c * UPC) // P
        lou = ((ru[lo_i:hi_i] - c * UPC) % P).astype(f)
        cnt = np.bincount(rw, minlength=UWIN)
        assert cnt.max() <= TU * P, f"core {c}: user window {cnt.max()}"
        rank = np.arange(n_u) - np.concatenate([[0], np.cumsum(cnt)])[rw]
        slot = rw * (TU * P) + rank
        col_arr = np.zeros(TUT * P, np.int32)
        val_arr = np.zeros(TUT * P, f)
        lou_arr = np.zeros(TUT * P, f)
        col_arr[slot] = cu[lo_i:hi_i]
        val_arr[slot] = vu[lo_i:hi_i]
        lou_arr[slot] = lou
        in_maps[c]["ucolT"] = np.ascontiguousarray(col_arr.reshape(TUT, P).T)
        in_maps[c]["uvalT"] = np.ascontiguousarray(val_arr.reshape(TUT, P).T)
        in_maps[c]["ulouT"] = np.ascontiguousarray(lou_arr.reshape(TUT, P).T)

    shared = {
        "emb": np.ascontiguousarray(np.asarray(entity_emb, f)),
        "wkv": wkv,
        "wq": np.ascontiguousarray(np.asarray(W_Q, f)),
        "wo": np.ascontiguousarray(np.asarray(W_O, f)),
    }
    for m in in_maps:
        m.update(shared)
    return in_maps, bases, widths


def kernel(entity_emb, W_Q, W_K, W_V, W_O, relation_att, relation_msg,
           relation_emb, edge_index, edge_type, im_rows, im_cols, im_vals,
           n_users, trace=False):
    from concourse.bass_utils import run_bass_kernel_spmd

    in_maps, bases, widths = _prep(
        np.asarray(entity_emb), np.asarray(W_Q), np.asarray(W_K),
        np.asarray(W_V), np.asarray(W_O), np.asarray(relation_att),
        np.asarray(relation_msg), np.asarray(edge_index),
        np.asarray(edge_type), np.asarray(im_rows), np.asarray(im_cols),
        np.asarray(im_vals))

    if "nc" not in _CACHE:
        _CACHE["nc"] = _build_program()
    nc = _CACHE["nc"]

    res = run_bass_kernel_spmd(nc, in_maps, list(range(NCORES)), trace=trace)
    _CACHE["last_result"] = res

    entity_agg = np.zeros((N, C), np.float32)
    user_agg = np.zeros((NU, C), np.float32)
    for c in range(NCORES):
        out = res.results[c]
        entity_agg[bases[c]:bases[c] + widths[c]] = \
            out["out_entT"][:, :widths[c]].T
        user_agg[c * UPC:(c + 1) * UPC] = out["out_user"][:UPC]
    return entity_agg, user_agg


# revision 8
# speedup vs baseline: 1.2356x; 1.2356x over previous
"""HGT layer kernel for Trainium2, 8 NeuronCores (SPMD via bass/Tile).

Strategy (per spec sharding hint): shard edges/nnz across 8 cores by
destination (entity for entity_agg, user for user_agg) so every segment
reduction completes on one core; replicate entity_emb and the small
weight/relation tensors.

Per core, three phases:
  A (relation-major): indirect-gather emb rows for tail/head, PE-transpose,
    q/k/v matmuls (relation matrices pre-folded into W_K/W_V block-diagonals
    on host), score -> exp, u' = [v*ex | ex] staged to HBM.
  B (destination-major): indirect-gather u' rows by slot, one-hot scatter
    matmul into 128-node PSUM windows (numerator + denominator together),
    then divide + W_O per window.
  U (user): indirect-gather emb rows, one-hot (x im_vals) scatter matmul
    into 128-user PSUM windows.
"""

import math

import numpy as np

# --- problem constants (hardcoded; kernel.py must be self-contained) ---
N = 100000
C = 128
E = 600000
NNZ = 1000000
NU = 50000
NREL = 24
H = 4
DK = 32
NCORES = 8

P = 128
G = 8                 # pass-A tiles per gather group
TR = 28               # pass-A tiles per relation (capacity 3584 edges)
TA = NREL * TR        # 672 pass-A tiles
EA = TA * P           # 86016 pass-A slots
NWIN = 100            # entity node windows per core (12800 nodes)
TW = 8                # pass-B tiles per node window (capacity 1024 edges)
TB = NWIN * TW        # 800 pass-B tiles
UPC = NU // NCORES    # 6250 users per core
UWIN = 49             # user windows per core (6272 users >= 6250)
TU = 23               # user tiles per window (capacity 2944 nnz)
TUT = UWIN * TU       # 1127 user tiles
UC = 132              # u' row: 128 weighted-v + 4 ex columns

_CACHE = {}


def _build_program():
    import os
    import concourse.bass as bass
    import concourse.tile as tile
    from concourse import bacc, mybir
    from concourse.masks import make_identity

    skip = set(os.environ.get("HGT_SKIP", "").split(","))

    f32 = mybir.dt.float32
    i32 = mybir.dt.int32

    nc = bacc.Bacc("TRN2", target_bir_lowering=False, debug=False,
                   num_devices=NCORES)

    emb = nc.dram_tensor("emb", [N, C], f32, kind="ExternalInput")
    wkv = nc.dram_tensor("wkv", [NREL * C, 2 * C], f32, kind="ExternalInput")
    wq = nc.dram_tensor("wq", [C, C], f32, kind="ExternalInput")
    wo = nc.dram_tensor("wo", [C, C], f32, kind="ExternalInput")
    tailT = nc.dram_tensor("tailT", [P, TA], i32, kind="ExternalInput")
    headT = nc.dram_tensor("headT", [P, TA], i32, kind="ExternalInput")
    biasT = nc.dram_tensor("biasT", [P, TA], f32, kind="ExternalInput")
    uslotT = nc.dram_tensor("uslotT", [P, TB], i32, kind="ExternalInput")
    lobT = nc.dram_tensor("lobT", [P, TB], f32, kind="ExternalInput")
    ucolT = nc.dram_tensor("ucolT", [P, TUT], i32, kind="ExternalInput")
    uvalT = nc.dram_tensor("uvalT", [P, TUT], f32, kind="ExternalInput")
    ulouT = nc.dram_tensor("ulouT", [P, TUT], f32, kind="ExternalInput")

    out_entT = nc.dram_tensor("out_entT", [P, NWIN * P], f32,
                              kind="ExternalOutput")
    out_user = nc.dram_tensor("out_user", [UWIN * P, C], f32,
                              kind="ExternalOutput")

    u_stage = nc.dram_tensor("u_stage", [EA, UC], f32)

    mult = mybir.AluOpType.mult
    addop = mybir.AluOpType.add
    iseq = mybir.AluOpType.is_equal
    AX = mybir.AxisListType.X
    EXP = mybir.ActivationFunctionType.Exp

    def bc(ap, n):
        """broadcast a [128, k] AP to [128, k, n] with a step-0 inner dim."""
        return bass.AP(ap.tensor, ap.offset, ap.ap + [[0, n]])

    with tile.TileContext(nc) as tc:
        with tc.tile_pool(name="const", bufs=1) as cpool:
            wkv_sb = cpool.tile([P, NREL * 2 * C], f32)
            for r in range(NREL):
                nc.sync.dma_start(wkv_sb[:, r * 2 * C:(r + 1) * 2 * C],
                                  wkv[r * C:(r + 1) * C, :])
            wq_sb = cpool.tile([P, C], f32)
            nc.sync.dma_start(wq_sb[:], wq[:])
            wo_sb = cpool.tile([P, C], f32)
            nc.sync.dma_start(wo_sb[:], wo[:])
            ident = cpool.tile([P, P], f32)
            make_identity(nc, ident[:])
            iota_i = cpool.tile([P, P], i32)
            nc.gpsimd.iota(iota_i[:], pattern=[[1, P]], base=0,
                           channel_multiplier=0)
            iota_f = cpool.tile([P, P], f32)
            nc.vector.tensor_copy(iota_f[:], iota_i[:])
            tailT_sb = cpool.tile([P, TA], i32)
            nc.sync.dma_start(tailT_sb[:], tailT[:])
            headT_sb = cpool.tile([P, TA], i32)
            nc.sync.dma_start(headT_sb[:], headT[:])
            biasT_sb = cpool.tile([P, TA], f32)
            nc.sync.dma_start(biasT_sb[:], biasT[:])
            uslotT_sb = cpool.tile([P, TB], i32)
            nc.sync.dma_start(uslotT_sb[:], uslotT[:])
            lobT_sb = cpool.tile([P, TB], f32)
            nc.sync.dma_start(lobT_sb[:], lobT[:])
            ucolT_sb = cpool.tile([P, TUT], i32)
            nc.sync.dma_start(ucolT_sb[:], ucolT[:])
            uvalT_sb = cpool.tile([P, TUT], f32)
            nc.sync.dma_start(uvalT_sb[:], uvalT[:])
            ulouT_sb = cpool.tile([P, TUT], f32)
            nc.sync.dma_start(ulouT_sb[:], ulouT[:])
            entT_sb = cpool.tile([P, NWIN * P], f32)

            scale = 1.0 / math.sqrt(DK)

            # ---------------- pass A ----------------
            with tc.tile_pool(name="pa_g", bufs=2) as pg, \
                 tc.tile_pool(name="pa_u", bufs=2) as pu, \
                 tc.tile_pool(name="pa_w", bufs=3) as pw, \
                 tc.tile_pool(name="pa_s", bufs=3) as ps, \
                 tc.tile_pool(name="pa_ps", bufs=2, space="PSUM") as pps, \
                 tc.tile_pool(name="pa_ps2", bufs=2, space="PSUM") as pps2:
                for g in range(0 if 'A' in skip else TA // G):
                    ug = pu.tile([P, G, UC], f32, tag="ug")
                    for j in range(G):
                        t = g * G + j
                        r = t // TR
                        gt = pg.tile([P, C], f32, tag="gt")
                        nc.gpsimd.indirect_dma_start(
                            out=gt[:], out_offset=None, in_=emb[:],
                            in_offset=bass.IndirectOffsetOnAxis(
                                ap=tailT_sb[:, t:t + 1], axis=0))
                        gh = pg.tile([P, C], f32, tag="gh")
                        nc.gpsimd.indirect_dma_start(
                            out=gh[:], out_offset=None, in_=emb[:],
                            in_offset=bass.IndirectOffsetOnAxis(
                                ap=headT_sb[:, t:t + 1], axis=0))
                        ptt = pps.tile([P, P], f32, tag="ptt")
                        nc.tensor.transpose(ptt[:], gt[:], ident[:])
                        ett = pw.tile([P, P], f32, tag="ett")
                        nc.scalar.copy(ett[:], ptt[:])
                        pth = pps.tile([P, P], f32, tag="pth")
                        nc.tensor.transpose(pth[:], gh[:], ident[:])
                        eth = pw.tile([P, P], f32, tag="eth")
                        nc.vector.tensor_copy(eth[:], pth[:])
                        pkv = pps2.tile([P, 2 * C], f32, tag="pkv")
                        nc.tensor.matmul(pkv[:], lhsT=ett[:],
                                         rhs=wkv_sb[:, r * 2 * C:(r + 1) * 2 * C],
                                         start=True, stop=True)
                        pq = pps2.tile([P, C], f32, tag="pq")
                        nc.tensor.matmul(pq[:], lhsT=eth[:], rhs=wq_sb[:],
                                         start=True, stop=True)
                        ksb = pw.tile([P, C], f32, tag="ksb")
                        nc.scalar.copy(ksb[:], pkv[:, 0:C])
                        qk = ps.tile([P, C], f32, tag="qk")
                        nc.vector.tensor_tensor(qk[:], pq[:], ksb[:], op=mult)
                        sc = ps.tile([P, H], f32, tag="sc")
                        nc.vector.tensor_reduce(
                            sc[:], qk[:].rearrange("p (h d) -> p h d", d=DK),
                            axis=AX, op=addop)
                        nc.scalar.activation(ug[:, j, C:UC], sc[:], EXP,
                                             bias=biasT_sb[:, t:t + 1],
                                             scale=scale)
                        nc.vector.tensor_tensor(
                            ug[:, j, 0:C].rearrange("p (h d) -> p h d", d=DK),
                            pkv[:, C:2 * C].rearrange("p (h d) -> p h d", d=DK),
                            bc(ug[:, j, C:UC], DK), op=mult)
                    nc.sync.dma_start(
                        u_stage[g * G * P:(g + 1) * G * P, :].rearrange(
                            "(j p) c -> p j c", p=P), ug[:])

            # ---------------- pass B ----------------
            with tc.tile_pool(name="pb_g", bufs=2) as pbg, \
                 tc.tile_pool(name="pb_s", bufs=3) as pbs, \
                 tc.tile_pool(name="pb_w", bufs=2) as pbw, \
                 tc.tile_pool(name="pb_ps", bufs=2, space="PSUM") as pbps, \
                 tc.tile_pool(name="pb_ps2", bufs=2, space="PSUM") as pbps2:
                for w in range(0 if 'B' in skip else NWIN):
                    ub = pbg.tile([P, TW, UC], f32, tag="ub")
                    for j in range(TW):
                        nc.gpsimd.indirect_dma_start(
                            out=ub[:, j, :], out_offset=None, in_=u_stage[:],
                            in_offset=bass.IndirectOffsetOnAxis(
                                ap=uslotT_sb[:, w * TW + j:w * TW + j + 1],
                                axis=0))
                    pwin = pbps.tile([P, UC], f32, tag="pwin")
                    for j in range(TW):
                        S = pbs.tile([P, P], f32, tag="S")
                        nc.vector.tensor_scalar(
                            S[:], iota_f[:],
                            lobT_sb[:, w * TW + j:w * TW + j + 1], None,
                            op0=iseq)
                        nc.tensor.matmul(pwin[:], lhsT=S[:], rhs=ub[:, j, :],
                                         start=(j == 0), stop=(j == TW - 1))
                    den = pbw.tile([P, H], f32, tag="den")
                    nc.vector.tensor_scalar(den[:], pwin[:, C:UC], 1e-30,
                                            None, op0=addop)
                    rec = pbw.tile([P, H], f32, tag="rec")
                    nc.vector.reciprocal(rec[:], den[:])
                    xn = pbw.tile([P, C], f32, tag="xn")
                    nc.vector.tensor_tensor(
                        xn[:].rearrange("p (h d) -> p h d", d=DK),
                        pwin[:, 0:C].rearrange("p (h d) -> p h d", d=DK),
                        bc(rec[:], DK), op=mult)
                    ptr = pbps2.tile([P, P], f32, tag="ptr")
                    nc.tensor.transpose(ptr[:], xn[:], ident[:])
                    xt = pbw.tile([P, C], f32, tag="xt")
                    nc.scalar.copy(xt[:], ptr[:])
                    po = pbps2.tile([P, C], f32, tag="po")
                    nc.tensor.matmul(po[:], lhsT=wo_sb[:], rhs=xt[:],
                                     start=True, stop=True)
                    nc.vector.tensor_copy(entT_sb[:, w * P:(w + 1) * P], po[:])
                nc.sync.dma_start(out_entT[:], entT_sb[:])

            # ---------------- user phase ----------------
            with tc.tile_pool(name="uu_g", bufs=2) as uug, \
                 tc.tile_pool(name="uu_s", bufs=3) as uus, \
                 tc.tile_pool(name="uu_o", bufs=2) as uuo, \
                 tc.tile_pool(name="uu_ps", bufs=2, space="PSUM") as uups:
                for w in range(0 if 'U' in skip else UWIN):
                    gu = uug.tile([P, TU, C], f32, tag="gu")
                    for j in range(TU):
                        nc.gpsimd.indirect_dma_start(
                            out=gu[:, j, :], out_offset=None, in_=emb[:],
                            in_offset=bass.IndirectOffsetOnAxis(
                                ap=ucolT_sb[:, w * TU + j:w * TU + j + 1],
                                axis=0))
                    pusr = uups.tile([P, C], f32, tag="pusr")
                    for j in range(TU):
                        S = uus.tile([P, P], f32, tag="Su")
                        nc.vector.tensor_scalar(
                            S[:], iota_f[:],
                            ulouT_sb[:, w * TU + j:w * TU + j + 1],
                            uvalT_sb[:, w * TU + j:w * TU + j + 1],
                            op0=iseq, op1=mult)
                        nc.tensor.matmul(pusr[:], lhsT=S[:], rhs=gu[:, j, :],
                                         start=(j == 0), stop=(j == TU - 1))
                    ou = uuo.tile([P, C], f32, tag="ou")
                    nc.vector.tensor_copy(ou[:], pusr[:])
                    nc.sync.dma_start(out_user[w * P:(w + 1) * P, :], ou[:])

    nc.compile()
    return nc


def _prep(entity_emb, W_Q, W_K, W_V, W_O, relation_att, relation_msg,
          edge_index, edge_type, im_rows, im_cols, im_vals):
    """Host-side sharding/index prep. Returns (in_maps, bases, widths)."""
    f = np.float32

    # fold relation matrices into block-diagonal combined weights
    bd_att = np.zeros((NREL, C, C), f)
    bd_msg = np.zeros((NREL, C, C), f)
    for h in range(H):
        s = h * DK
        bd_att[:, s:s + DK, s:s + DK] = relation_att[:, h]
        bd_msg[:, s:s + DK, s:s + DK] = relation_msg[:, h]
    wk_r = np.einsum("ck,rkd->rcd", W_K.astype(f), bd_att)
    wv_r = np.einsum("ck,rkd->rcd", W_V.astype(f), bd_msg)
    wkv = np.concatenate([wk_r, wv_r], axis=2).reshape(NREL * C, 2 * C)
    wkv = np.ascontiguousarray(wkv)

    head = np.asarray(edge_index[0], np.int64)
    tail = np.asarray(edge_index[1], np.int64)
    rel = np.asarray(edge_type, np.int64) - 1
    order = np.argsort(head, kind="stable")
    hs, ts, rs = head[order], tail[order], rel[order]

    # core boundaries at node boundaries, balancing edge counts
    bases, splits = [0], [0]
    for c in range(1, NCORES):
        b = int(hs[min(c * (E // NCORES), E - 1)])
        bases.append(b)
        splits.append(int(np.searchsorted(hs, b, side="left")))
    bases.append(N)
    splits.append(E)

    in_maps = []
    widths = []
    for c in range(NCORES):
        lo_e, hi_e = splits[c], splits[c + 1]
        base = bases[c]
        width = bases[c + 1] - base
        widths.append(width)
        n_c = hi_e - lo_e
        ch, ct, cr = hs[lo_e:hi_e], ts[lo_e:hi_e], rs[lo_e:hi_e]
        assert width <= NWIN * P, f"core {c}: node width {width}"
        assert n_c <= EA

        # pass-A slots: group by relation
        cnt_r = np.bincount(cr, minlength=NREL)
        assert cnt_r.max() <= TR * P - 1, f"core {c}: rel count {cnt_r.max()}"
        ord_r = np.argsort(cr, kind="stable")
        rank_r = np.empty(n_c, np.int64)
        rank_r[ord_r] = np.arange(n_c) - np.concatenate(
            [[0], np.cumsum(cnt_r)])[cr[ord_r]]
        slot_a = cr * (TR * P) + rank_r

        tail_a = np.zeros(EA, np.int32)
        head_a = np.zeros(EA, np.int32)
        bias_a = np.full(EA, -1e30, f)
        tail_a[slot_a] = ct
        head_a[slot_a] = ch
        bias_a[slot_a] = 0.0

        # pass-B slots: group by 128-node window (edges already head-sorted)
        wloc = (ch - base) // P
        lob = ((ch - base) % P).astype(f)
        cnt_w = np.bincount(wloc, minlength=NWIN)
        assert cnt_w.max() <= TW * P, f"core {c}: window count {cnt_w.max()}"
        rank_w = np.arange(n_c) - np.concatenate([[0], np.cumsum(cnt_w)])[wloc]
        slot_b = wloc * (TW * P) + rank_w

        pad_slot = int(cnt_r[0])  # a pass-A padding slot (ex == 0 there)
        slot_arr = np.full(TB * P, pad_slot, np.int32)
        lob_arr = np.zeros(TB * P, f)
        slot_arr[slot_b] = slot_a
        lob_arr[slot_b] = lob

        in_maps.append({
            "tailT": np.ascontiguousarray(tail_a.reshape(TA, P).T),
            "headT": np.ascontiguousarray(head_a.reshape(TA, P).T),
            "biasT": np.ascontiguousarray(bias_a.reshape(TA, P).T),
            "uslotT": np.ascontiguousarray(slot_arr.reshape(TB, P).T),
            "lobT": np.ascontiguousarray(lob_arr.reshape(TB, P).T),
        })

    # user phase
    rows = np.asarray(im_rows, np.int64)
    cols = np.asarray(im_cols, np.int64)
    vals = np.asarray(im_vals, f)
    order_u = np.argsort(rows, kind="stable")
    ru, cu, vu = rows[order_u], cols[order_u], vals[order_u]
    for c in range(NCORES):
        lo_i = int(np.searchsorted(ru, c * UPC, side="left"))
        hi_i = int(np.searchsorted(ru, (c + 1) * UPC, side="left"))
        n_u = hi_i - lo_i
        rw = (ru[lo_i:hi_i] - c * UPC) // P
        lou = ((ru[lo_i:hi_i] - c * UPC) % P).astype(f)
        cnt = np.bincount(rw, minlength=UWIN)
        assert cnt.max() <= TU * P, f"core {c}: user window {cnt.max()}"
        rank = np.arange(n_u) - np.concatenate([[0], np.cumsum(cnt)])[rw]
        slot = rw * (TU * P) + rank
        col_arr = np.zeros(TUT * P, np.int32)
        val_arr = np.zeros(TUT * P, f)
        lou_arr = np.zeros(TUT * P, f)
        col_arr[slot] = cu[lo_i:hi_i]
        val_arr[slot] = vu[lo_i:hi_i]
        lou_arr[slot] = lou
        in_maps[c]["ucolT"] = np.ascontiguousarray(col_arr.reshape(TUT, P).T)
        in_maps[c]["uvalT"] = np.ascontiguousarray(val_arr.reshape(TUT, P).T)
        in_maps[c]["ulouT"] = np.ascontiguousarray(lou_arr.reshape(TUT, P).T)

    shared = {
        "emb": np.ascontiguousarray(np.asarray(entity_emb, f)),
        "wkv": wkv,
        "wq": np.ascontiguousarray(np.asarray(W_Q, f)),
        "wo": np.ascontiguousarray(np.asarray(W_O, f)),
    }
    for m in in_maps:
        m.update(shared)
    return in_maps, bases, widths


def kernel(entity_emb, W_Q, W_K, W_V, W_O, relation_att, relation_msg,
           relation_emb, edge_index, edge_type, im_rows, im_cols, im_vals,
           n_users, trace=False):
    from concourse.bass_utils import run_bass_kernel_spmd

    in_maps, bases, widths = _prep(
        np.asarray(entity_emb), np.asarray(W_Q), np.asarray(W_K),
        np.asarray(W_V), np.asarray(W_O), np.asarray(relation_att),
        np.asarray(relation_msg), np.asarray(edge_index),
        np.asarray(edge_type), np.asarray(im_rows), np.asarray(im_cols),
        np.asarray(im_vals))

    if "nc" not in _CACHE:
        _CACHE["nc"] = _build_program()
    nc = _CACHE["nc"]

    res = run_bass_kernel_spmd(nc, in_maps, list(range(NCORES)), trace=trace)
    _CACHE["last_result"] = res

    entity_agg = np.zeros((N, C), np.float32)
    user_agg = np.zeros((NU, C), np.float32)
    for c in range(NCORES):
        out = res.results[c]
        entity_agg[bases[c]:bases[c] + widths[c]] = \
            out["out_entT"][:, :widths[c]].T
        user_agg[c * UPC:(c + 1) * UPC] = out["out_user"][:UPC]
    return entity_agg, user_agg
